# revision 48
# baseline (speedup 1.0000x reference)
"""Trainium2 Bass kernel for AttributeAttentionModule.

y = attention over heads of QKV projections:
  Q = sa @ Wq.T + bq ; K = x @ Wk.T + bk ; V = x @ Wv.T + bv   (all [B, D])
  per-sample scores[h,g] = Q_h . K_g / 32 ; softmax over g ; out_h = sum_g w_hg V_g

Data-parallel over 8 NeuronCores (batch sharded). Q/K projections run in
fp8-e4m3 with MatmulPerfMode.DoubleRow (2 contraction tiles per matmul;
the per-sample softmax tolerates the quantization), V in bf16. All inputs
are pre-tiled and pre-quantized on the host so every DMA descriptor is a
contiguous block; scales (32 for activations, 8192 for weights) keep every
fp8 value under 240 so e4m3/e4m3fn encodings agree. PSUM accumulates in
fp32 and the descale is folded into the PSUM->SBUF bias-add. Weights
stream on the SP DMA queue, activations on Pool, spills/attention on ACT,
so no stream blocks another. Attention is software-pipelined into the
matmul stream as load/score/combine chunks drained between o-sweeps
(scores on DVE, combines split across Pool and ACT).
"""

import os
import sys

for _p in ("/opt/trn_rl_repo", "/root/.axon_site/_ro/trn_rl_repo"):
    if os.path.isdir(_p) and _p not in sys.path:
        sys.path.append(_p)

import numpy as np
from contextlib import ExitStack

B = 16384
D = 3072
H = 3
DH = D // H          # 1024
NCORES = 8
P = 128              # partition tile
NO = 512             # matmul moving free dim (one PSUM bank of fp32)
KT = D // P          # 24 contraction tiles
NOT = D // NO        # 6 output-column tiles
KGRP8 = 12           # k-tiles per fp8 weight DMA
NKG8 = KT // KGRP8   # 2 fp8 weight DMAs per o-column
KGRPV = 8            # k-tiles per bf16 weight DMA
NKGV = KT // KGRPV   # 3 bf16 weight DMAs per o-column
KHALF = KT // 2      # stationary tiles arrive in two halves
SA = 32.0            # fp8 activation scale
SW = 8192.0          # fp8 weight scale
DESCALE = 1.0 / (SA * SW)

_CACHE = {}


def _build(bs=B // NCORES, gbt=8):
    """Build + compile the per-core program. bs = batch rows per core,
    gbt = batch tiles (of 128) per weight-streaming group."""
    import concourse.bass as bass
    import concourse.tile as tile
    from concourse import bacc, mybir

    f32 = mybir.dt.float32
    bf16 = mybir.dt.bfloat16
    f8 = mybir.dt.float8e4
    DR = mybir.MatmulPerfMode.DoubleRow
    mult = mybir.AluOpType.mult
    add = mybir.AluOpType.add
    bypass = mybir.AluOpType.bypass
    Exp = mybir.ActivationFunctionType.Exp

    nbt = bs // P        # batch tiles per core
    ng = nbt // gbt      # weight-stream groups

    nc = bacc.Bacc(
        "TRN2", target_bir_lowering=False, debug=False, num_devices=NCORES
    )

    # pre-tiled inputs (see kernel() for host layouts)
    sa8d = nc.dram_tensor("sa8", [nbt, P, KT, P], f8, kind="ExternalInput").ap()
    x8d = nc.dram_tensor("x8", [nbt, P, KT, P], f8, kind="ExternalInput").ap()
    xbd = nc.dram_tensor("xb", [nbt, P, KT, P], bf16, kind="ExternalInput").ap()
    wq8 = nc.dram_tensor(
        "wq8", [NOT, NKG8, P, KGRP8, NO], f8, kind="ExternalInput"
    ).ap()
    wk8 = nc.dram_tensor(
        "wk8", [NOT, NKG8, P, KGRP8, NO], f8, kind="ExternalInput"
    ).ap()
    wv5 = nc.dram_tensor(
        "wv5", [NOT, NKGV, P, KGRPV, NO], bf16, kind="ExternalInput"
    ).ap()
    biasd = {
        t: nc.dram_tensor(f"b{t}", [P, D], bf16, kind="ExternalInput").ap()
        for t in "qkv"
    }
    outd = nc.dram_tensor("out", [bs, D], bf16, kind="ExternalOutput").ap()

    with tile.TileContext(nc) as tc, ExitStack() as ctx:
        dram = ctx.enter_context(tc.tile_pool(name="dram", bufs=1, space="DRAM"))
        qkv_s = {
            t: dram.tile([bs, D], bf16, tag=f"s{t}", name=f"s{t}") for t in "qkv"
        }

        apool = ctx.enter_context(tc.tile_pool(name="apool", bufs=1))
        wp8 = ctx.enter_context(tc.tile_pool(name="wp8", bufs=3))
        wpv = ctx.enter_context(tc.tile_pool(name="wpv", bufs=2))
        bpool = ctx.enter_context(tc.tile_pool(name="bpool", bufs=1))
        ocpool = ctx.enter_context(tc.tile_pool(name="ocpool", bufs=3))
        pspool = ctx.enter_context(tc.tile_pool(name="psum", bufs=1, space="PSUM"))
        qkvp = ctx.enter_context(tc.tile_pool(name="qkvp", bufs=2))
        smallp = ctx.enter_context(tc.tile_pool(name="smallp", bufs=8))
        accp = ctx.enter_context(tc.tile_pool(name="accp", bufs=2))
        prodp = ctx.enter_context(tc.tile_pool(name="prodp", bufs=1))
        outp = ctx.enter_context(tc.tile_pool(name="outp", bufs=2))

        pending = []  # attention chunk closures, drained between o-sweeps

        def filler():
            if pending:
                pending.pop(0)()

        def load_act(src, g, tg, dt):
            """Two half-k tiles per batch tile so matmuls can start on the
            first half while the second streams in. Distinct tags per
            stream so later streams prefetch during earlier projections."""
            los, his = [], []
            for i in range(gbt):
                lo = apool.tile([P, KHALF, P], dt, tag=f"{tg}{i}l", name=f"{tg}{i}l")
                nc.gpsimd.dma_start(lo[:], src[g * gbt + i, :, 0:KHALF, :])
                los.append(lo)
            for i in range(gbt):
                hi = apool.tile([P, KHALF, P], dt, tag=f"{tg}{i}h", name=f"{tg}{i}h")
                nc.gpsimd.dma_start(hi[:], src[g * gbt + i, :, KHALF:KT, :])
                his.append(hi)
            return list(zip(los, his))

        def proj8(items, wTd, bias_t, dst):
            """fp8 DoubleRow projection: each matmul consumes two k-tiles.
            items: list of (global_bt_index, (a_lo, a_hi))."""
            for o in range(NOT):
                ps = {
                    bt: pspool.tile([P, NO], f32, tag=f"ps{j}", name=f"ps{j}")
                    for j, (bt, _) in enumerate(items)
                }
                for kg in range(NKG8):
                    wt = wp8.tile([P, KGRP8, NO], f8, tag="w8", name="w8")
                    nc.sync.dma_start(wt[:], wTd[o, kg])
                    for jj in range(0, KGRP8, 2):
                        k = kg * KGRP8 + jj
                        for bt, (alo, ahi) in items:
                            a = alo if k < KHALF else ahi
                            ka = k % KHALF
                            nc.tensor.matmul(
                                ps[bt][:],
                                a[:, ka : ka + 2, :],
                                wt[:, jj : jj + 2, :],
                                start=(k == 0),
                                stop=(k == KT - 2),
                                perf_mode=DR,
                            )
                for bt, _ in items:
                    oc = ocpool.tile([P, NO], bf16, tag="oc", name="oc")
                    # descale fp8 product and add bias in one DVE op
                    nc.vector.scalar_tensor_tensor(
                        oc[:],
                        ps[bt][:],
                        DESCALE,
                        bias_t[:, o * NO : (o + 1) * NO],
                        op0=mult,
                        op1=add,
                    )
                    nc.scalar.dma_start(
                        dst[bt * P : bt * P + P, o * NO : (o + 1) * NO], oc[:]
                    )
                filler()

        def proj(items, wTd, bias_t, dst):
            """bf16 projection. items: list of (global_bt_index, (a_lo, a_hi))."""
            for o in range(NOT):
                ps = {
                    bt: pspool.tile([P, NO], f32, tag=f"ps{j}", name=f"ps{j}")
                    for j, (bt, _) in enumerate(items)
                }
                for kg in range(NKGV):
                    wt = wpv.tile([P, KGRPV, NO], bf16, tag="wv", name="wv")
                    nc.sync.dma_start(wt[:], wTd[o, kg])
                    for j in range(KGRPV):
                        k = kg * KGRPV + j
                        for bt, (alo, ahi) in items:
                            a = alo if k < KHALF else ahi
                            nc.tensor.matmul(
                                ps[bt][:],
                                a[:, k % KHALF, :],
                                wt[:, j, :],
                                start=(k == 0),
                                stop=(k == KT - 1),
                            )
                    filler()
                for bt, _ in items:
                    oc = ocpool.tile([P, NO], bf16, tag="oc", name="oc")
                    nc.vector.tensor_add(
                        oc[:], ps[bt][:], bias_t[:, o * NO : (o + 1) * NO]
                    )
                    nc.scalar.dma_start(
                        dst[bt * P : bt * P + P, o * NO : (o + 1) * NO], oc[:]
                    )
                filler()

        def attn_chunks(bt):
            """Chunks per batch tile: load Q/K rows, three score sub-chunks
            (scores only need Q/K, so they drain during the V projection;
            small pieces keep the DVE queue responsive for PSUM copies),
            load V rows, weighted V combine + store."""
            r0 = bt * P
            t3 = {}
            small = {}

            def c_load_qk():
                for t in "qk":
                    tt = qkvp.tile([P, D], bf16, tag=t, name=f"t_{t}")
                    nc.scalar.dma_start(tt[:], qkv_s[t][r0 : r0 + P, :])
                    t3[t] = tt

            def c_load_v():
                tt = qkvp.tile([P, D], bf16, tag="v", name="t_v")
                nc.scalar.dma_start(tt[:], qkv_s["v"][r0 : r0 + P, :])
                t3["v"] = tt

            def dots(hgs):
                prod = prodp.tile([P, DH], bf16, tag="prod", name="prod")
                for h, g2 in hgs:
                    # fused row-wise dot: prod = Q_h*K_g ; s_hg = sum(prod)
                    nc.vector.scalar_tensor_tensor(
                        prod[:],
                        t3["q"][:, h * DH : (h + 1) * DH],
                        1.0,
                        t3["k"][:, g2 * DH : (g2 + 1) * DH],
                        op0=bypass,
                        op1=mult,
                        accum_out=small["s"][:, h * H + g2 : h * H + g2 + 1],
                    )

            def c_score_a():
                small["s"] = smallp.tile([P, H * H], f32, tag="s", name="s")
                dots([(0, 0), (0, 1), (0, 2)])

            def c_score_b():
                dots([(1, 0), (1, 1), (1, 2)])

            def c_score_c():
                dots([(2, 0), (2, 1), (2, 2)])
                s = small["s"]
                e = smallp.tile([P, H * H], f32, tag="e", name="e")
                nc.scalar.activation(e[:], s[:], Exp, scale=1.0 / 32.0)
                ssum = smallp.tile([P, H], f32, tag="ssum", name="ssum")
                nc.vector.tensor_reduce(
                    ssum[:],
                    e[:].rearrange("p (h g) -> p h g", h=H),
                    axis=mybir.AxisListType.X,
                    op=add,
                )
                rcp = smallp.tile([P, H], f32, tag="rcp", name="rcp")
                nc.vector.reciprocal(rcp[:], ssum[:])
                # pre-normalized softmax weights: shortens the combine chain
                wn = smallp.tile([P, H * H], f32, tag="wn", name="wn")
                for h in range(H):
                    nc.scalar.mul(
                        wn[:, h * H : (h + 1) * H],
                        e[:, h * H : (h + 1) * H],
                        rcp[:, h : h + 1],
                    )
                small["wn"] = wn

            def comb_h(h):
                wn, ot = small["wn"], small["ot"]
                acc = accp.tile([P, DH], bf16, tag="acc", name="acc")
                # first term on ScalarE (per-partition scalar scale)
                nc.scalar.mul(acc[:], t3["v"][:, 0:DH], wn[:, h * H : h * H + 1])
                nc.vector.scalar_tensor_tensor(
                    acc[:],
                    t3["v"][:, DH : 2 * DH],
                    wn[:, h * H + 1 : h * H + 2],
                    acc[:],
                    op0=mult,
                    op1=add,
                )
                nc.vector.scalar_tensor_tensor(
                    ot[:, h * DH : (h + 1) * DH],
                    t3["v"][:, 2 * DH : 3 * DH],
                    wn[:, h * H + 2 : h * H + 3],
                    acc[:],
                    op0=mult,
                    op1=add,
                )

            def c_comb_a():
                small["ot"] = outp.tile([P, D], bf16, tag="o", name="o")
                comb_h(0)

            def c_comb_b():
                comb_h(1)
                comb_h(2)
                nc.scalar.dma_start(outd[r0 : r0 + P, :], small["ot"][:])

            return (c_load_qk, [c_score_a, c_score_b, c_score_c]), (
                c_load_v,
                [c_comb_a, c_comb_b],
            )

        bias_t = {}
        for t in "qkv":
            bias_t[t] = bpool.tile([P, D], bf16, tag=f"bias{t}", name=f"bias{t}")
            # ACT queue: empty at start, so these don't delay the first
            # weight tiles on the SP queue
            nc.scalar.dma_start(bias_t[t][:], biasd[t][:])

        def stagger(parts):
            """parts: list of (load, [computes]) pairs. Emit each load one
            batch-tile ahead of its compute chunks."""
            out = []
            prev = None
            for ld, comps in parts:
                out.append(ld)
                if prev:
                    out.extend(prev)
                prev = comps
            if prev:
                out.extend(prev)
            return out

        for g in range(ng):
            last = g == ng - 1
            bts = [g * gbt + i for i in range(gbt)]
            sa_t = load_act(sa8d, g, "s", f8)
            x_t = load_act(x8d, g, "x", f8)
            xb_t = load_act(xbd, g, "v", bf16)
            chunks = {}
            proj8(list(zip(bts, sa_t)), wq8, bias_t["q"], qkv_s["q"])
            proj8(list(zip(bts, x_t)), wk8, bias_t["k"], qkv_s["k"])
            for bt in bts:
                chunks[bt] = attn_chunks(bt)
            # scores only need Q/K: drain them during the V projection
            pending.extend(stagger([chunks[bt][0] for bt in bts]))
            items = list(zip(bts, xb_t))
            if last and gbt >= 2:
                half = max(gbt // 2, gbt - 3)  # small final slice -> short tail
                proj(items[:half], wv5, bias_t["v"], qkv_s["v"])
                pending.extend(stagger([chunks[bt][1] for bt in bts[:half]]))
                proj(items[half:], wv5, bias_t["v"], qkv_s["v"])
                pending.extend(stagger([chunks[bt][1] for bt in bts[half:]]))
            else:
                proj(items, wv5, bias_t["v"], qkv_s["v"])
                pending.extend(stagger([chunks[bt][1] for bt in bts]))
        while pending:
            pending.pop(0)()

    nc.compile()
    return nc


def _get_nc(bs=B // NCORES, gbt=8):
    key = (bs, gbt)
    if key not in _CACHE:
        _CACHE[key] = _build(bs, gbt)
    return _CACHE[key]


def _tile_w(W, nkg, kgrp):
    """w5[o, kg, p, j, n] = W.T[(kg*kgrp+j)*P + p, o*NO + n]."""
    wt = np.asarray(W, dtype=np.float32).T  # [in, out]
    return wt.reshape(nkg, kgrp, P, NOT, NO).transpose(3, 0, 2, 1, 4)


def _prep_weights(Wq, Wk, Wv, bq, bk, bv):
    import ml_dtypes

    bf16 = ml_dtypes.bfloat16
    f8 = ml_dtypes.float8_e4m3
    ws = {
        "q": np.ascontiguousarray((_tile_w(Wq, NKG8, KGRP8) * SW).astype(f8)),
        "k": np.ascontiguousarray((_tile_w(Wk, NKG8, KGRP8) * SW).astype(f8)),
        "v": np.ascontiguousarray(_tile_w(Wv, NKGV, KGRPV).astype(bf16)),
    }
    bb = {
        nm: np.ascontiguousarray(
            np.broadcast_to(np.asarray(b, dtype=np.float32), (P, D)).astype(bf16)
        )
        for nm, b in (("q", bq), ("k", bk), ("v", bv))
    }
    return ws, bb


def _prep_act(a, bs, dt, scale=1.0):
    """Pre-tile activations per core: a4[bt, p, ko, b] = a[bt*P + b, ko*P + p]."""
    nbt = bs // P
    a4 = a.reshape(nbt, P, KT, P).transpose(0, 3, 2, 1)
    if scale != 1.0:
        a4 = a4 * scale
    return np.ascontiguousarray(a4.astype(dt))


def _in_maps(x, sa, ws, bb, bs):
    import ml_dtypes

    bf16 = ml_dtypes.bfloat16
    f8 = ml_dtypes.float8_e4m3
    maps = []
    for c in range(NCORES):
        r0 = c * bs
        xs = x[r0 : r0 + bs]
        maps.append(
            {
                "sa8": _prep_act(sa[r0 : r0 + bs], bs, f8, SA),
                "x8": _prep_act(xs, bs, f8, SA),
                "xb": _prep_act(xs, bs, bf16),
                "wq8": ws["q"],
                "wk8": ws["k"],
                "wv5": ws["v"],
                "bq": bb["q"],
                "bk": bb["k"],
                "bv": bb["v"],
            }
        )
    return maps


def kernel(x, synthetic_attributes, Wq, bq, Wk, bk, Wv, bv, **_ignored):
    from concourse import bass_utils

    x = np.asarray(x, dtype=np.float32)
    sa = np.asarray(synthetic_attributes, dtype=np.float32)
    bs = x.shape[0] // NCORES

    ws, bb = _prep_weights(Wq, Wk, Wv, bq, bk, bv)
    nc = _get_nc(bs=bs)
    in_maps = _in_maps(x, sa, ws, bb, bs)

    res = bass_utils.run_bass_kernel_spmd(nc, in_maps, core_ids=list(range(NCORES)))
    out = np.concatenate([res.results[c]["out"] for c in range(NCORES)], axis=0)
    return np.ascontiguousarray(out.astype(np.float32))
